# revision 8
# baseline (speedup 1.0000x reference)
"""Trainium2 Bass kernel for nn_CausalDerivative (per-node MLP stack).

Computation (reference):
    x = where(discrete_mask, (inputs > 0), inputs)          # straight-through gate
    W1m = W1 * M[:, None, :]   (M = adjacency, last row one-hot @ last col)
    h = relu(einsum('bn,ihn->bih', x, W1m))                 # [B, N, H]
    out = einsum('bih,ih->bi', h, W2)                       # [B, N]

Strategy: pure data-parallel over 8 NeuronCores (batch sharded 4096/core).

Per core the kernel is paced by the relu eviction of the 16.8M-element z
intermediate (PSUM f32 -> SBUF bf16); on TRN2 only DVE (0.96 GHz) and ACT
(1.2 GHz) can read PSUM at 1 f32/cycle/lane, so ~70us/core is the floor.
Design (v2, ring conveyor):

  - straight-through gate and weight folding host-side; |W2| folded into
    W1 so stage-2 weights are signs; the sign matrix w2s is built ON CHIP
    (memset + 4 strided copies from a 16KB compact DMA) instead of a
    256KB DMA of mostly zeros.
  - stage 1: per 128-unit chunk, 2 block-diagonal K=64/M=128 matmuls
    (tile rows 0/64) each fill ONE PSUM bank [128u, 512b] -- half the MM
    and LDWEIGHTS instructions of the 4-quadrant scheme.
  - PSUM is a 7-bank ring (the 8th bank is the single-buffered stage-2
    accumulator).  Chunk g occupies banks (2g mod 7, 2g+1 mod 7); the
    eviction is ONE strided 2D-AP op over the pair (wrap pair (6,0) is
    filled A->0, B->6 so the AP stays ascending).  Ring depth 3.5 chunks
    decouples fills from the evict critical cycle (vs 3 slots before).
  - evictions alternate DVE / ACT per chunk; first two chunks split
    across both engines to start the conveyor early.
  - ~12 warm-up matmuls on a memset scratch trip the PE HAM throttle
    (cold 1.2GHz -> warm 2.4GHz) before the real fills arrive.
  - DMA: first wave = exactly k-tile 0's needs on 3 queues; w1 then
    streams in progressive column pieces so chunk j never waits on a
    monolithic transfer; sx second half last (needed 16 k-tiles in).
  - stage 2: per chunk pair, 4 concurrent K=128/M=32 col-strip matmuls
    accumulate into the acc bank over 16 pairs; acc -> SBUF copy on ACT
    (split DVE/ACT for the last tile) then DMA out.
  - node rows come out stripe-permuted; the host unpermutes.
"""

import os
import numpy as np

import concourse.bass as bass
import concourse.tile as tile
from concourse import mybir, bacc
from concourse.bass import ts
from concourse.bass_utils import run_bass_kernel_spmd

B, N, H = 32768, 64, 64
IH = N * H                    # 4096 hidden units total
N_CORES = 8
BL = B // N_CORES             # 4096 batch rows per core
HALF = BL // 2                # 2048 (batch half per SBUF partition group)
BW = 512                      # batch tile width (PE moving free dim)
NPAIR = HALF // BW            # 4 batch tiles per core
NCHUNK = IH // 128            # 32 ih chunks of 128 units (2 nodes each)
NK = NCHUNK // 2              # 16 chunk-pairs (k-tiles) per batch tile
RING = 7                      # PSUM ring banks (bank 8 = stage-2 acc)
NWARM = 12                    # HAM warm-up matmuls

F32 = mybir.dt.float32
BF16 = mybir.dt.bfloat16
DT = BF16
import ml_dtypes
NP_DT = ml_dtypes.bfloat16

LAST_EXEC_NS = None

_compiled = {}


def _ring_banks(g):
    """Chunk g -> (bankA, bankB, dynslice over the pair)."""
    base = (2 * g) % RING
    if base <= RING - 2:
        return base, base + 1, bass.ds(base, 2)
    # wrap: fill A into bank 0, B into bank 6 so ds(0,2,step=6) reads A|B
    return 0, RING - 1, bass.ds(0, 2, RING - 1)


def _build_module():
    """Emit the per-core Bass module (same program for all 8 cores)."""
    nc = bacc.Bacc("TRN2", target_bir_lowering=False, debug=False)
    xt = nc.dram_tensor("xt", [N, BL], DT, kind="ExternalInput").ap()
    w1 = nc.dram_tensor("w1", [N, IH], DT, kind="ExternalInput").ap()
    w2c = nc.dram_tensor("w2c", [128, 64], DT, kind="ExternalInput").ap()
    out = nc.dram_tensor("out", [N, BL], DT, kind="ExternalOutput").ap()

    DELAY = 3                 # stage-2 lags stage-1 by DELAY k-tiles
    NKT = NPAIR * NK          # 64 k-tiles total

    with tile.TileContext(nc) as tc:
        with (
            tc.tile_pool(name="consts", bufs=1) as consts,
            tc.tile_pool(name="hp", bufs=10) as hp,
            tc.tile_pool(name="so", bufs=2) as sop,
            tc.tile_pool(name="ring", bufs=1, space="PSUM") as ringp,
            tc.tile_pool(name="accs", bufs=1, space="PSUM") as accs,
        ):
            sx = consts.tile([128, HALF], DT)
            w1s = consts.tile([128, IH], DT)
            w2s = consts.tile([128, NCHUNK * 32], DT)
            w2stage = consts.tile([128, 64], DT)
            warm = consts.tile([64, 192], DT)

            ring = ringp.tile([128, RING, 512], F32)

            # ---- HAM warm-up: keep PE busy from t=0 so the clock gate
            # releases (1.2 -> 2.4 GHz) before the real fills arrive.
            nc.gpsimd.memset(warm[:], 0.0)
            # scratch output in ring bank 6 (first real use is chunk 3,
            # well after the warm-up completes)
            for _ in range(NWARM):
                nc.tensor.matmul(ring[0:64, RING - 1, 0:128], warm[0:64, 0:64],
                                 warm[0:64, 64:192])

            # ---- startup DMA.  Wave 1: exactly what chunk 0 needs.
            nc.scalar.dma_start(w1s[0:64, 0:256], w1[:, 0:256])
            nc.scalar.dma_start(w1s[64:128, 0:256], w1[:, 0:256])
            nc.sync.dma_start(sx[0:64, 0:BW], xt[:, 0:BW])
            nc.gpsimd.dma_start(sx[64:128, 0:BW], xt[:, HALF : HALF + BW])
            nc.scalar.dma_start(w2stage[:, :], w2c[:, :])
            # Wave 2: w1 in progressive 512-col pieces (A half on sync, B half
            # on gpsimd) so chunk j never waits on a monolithic transfer.
            w1cuts = [256] + list(range(768, IH, 512)) + [IH]
            for i in range(len(w1cuts) - 1):
                c0, c1 = w1cuts[i], w1cuts[i + 1]
                nc.sync.dma_start(w1s[0:64, c0:c1], w1[:, c0:c1])
                nc.gpsimd.dma_start(w1s[64:128, c0:c1], w1[:, c0:c1])
                if i == 1:
                    # zero the sign matrix once the early triggers are posted
                    nc.gpsimd.memset(w2s[:, :], 0.0)
            # Wave 3: rest of sx (needed from k-tile 16 onward).
            sxcuts = [BW, 1280, HALF]
            for i in range(len(sxcuts) - 1):
                c0, c1 = sxcuts[i], sxcuts[i + 1]
                nc.sync.dma_start(sx[0:64, c0:c1], xt[:, c0:c1])
                nc.gpsimd.dma_start(sx[64:128, c0:c1], xt[:, HALF + c0 : HALF + c1])
            # Build the sparse stage-2 sign matrix on chip: 4 strided
            # copies scatter the compact class-major signs into w2s.
            for pbit in range(2):
                for u in range(2):
                    cls = 2 * pbit + u
                    nc.gpsimd.tensor_copy(
                        w2s[64 * u : 64 * u + 64, bass.ds(32 * pbit + u, 16, 66)],
                        w2stage[64 * u : 64 * u + 64, bass.ds(16 * cls, 16)],
                    )

            hq = {}
            accq = {}

            def _evict_engine(g):
                """DVE for even chunks, ACT for odd -- except wrap-pair
                chunks (banks {0,6}, a strided AP only DVE handles) always
                go to DVE, swapping with the neighbouring even chunk so the
                64/64 engine split is preserved."""
                wrap = (2 * g) % RING == RING - 1
                if wrap:
                    return "D"
                if g % 2 == 0:
                    # even chunk whose odd neighbour is a wrap -> take ACT
                    if (2 * (g + 1)) % RING == RING - 1:
                        return "A"
                    return "D"
                return "A"

            def stage1(kt):
                p, t = divmod(kt, NK)
                bs = ts(p, BW)
                for u2 in range(2):
                    g = 2 * kt + u2           # global chunk index
                    jj = 2 * t + u2           # chunk within batch tile
                    bA, bB, sl = _ring_banks(g)
                    cw = bass.ds(128 * jj, 128)
                    nc.tensor.matmul(ring[:, bA, :], w1s[0:64, cw],
                                     sx[0:64, bs], tile_position=(0, 0))
                    nc.tensor.matmul(ring[:, bB, :], w1s[64:128, cw],
                                     sx[64:128, bs], tile_position=(64, 0))
                    h = hp.tile([128, 2 * BW], DT)
                    wrap = bB == RING - 1 and bA == 0
                    if wrap:
                        src3 = ring[:, sl, :]
                        dst3 = h[:].rearrange("p (a b) -> p a b", a=2)
                        nc.vector.tensor_scalar_max(dst3, src3, 0.0)
                    else:
                        src = ring[:, sl, :].rearrange("p a b -> p (a b)")
                        if g < 2:
                            # conveyor start: split across both engines
                            nc.vector.tensor_scalar_max(h[:, 0:BW],
                                                        ring[:, bA, :], 0.0)
                            nc.scalar.activation(h[:, BW : 2 * BW],
                                                 ring[:, bB, :],
                                                 mybir.ActivationFunctionType.Relu)
                        elif _evict_engine(g) == "D":
                            nc.vector.tensor_scalar_max(h[:], src, 0.0)
                        else:
                            nc.scalar.activation(h[:], src,
                                                 mybir.ActivationFunctionType.Relu)
                    hq[(p, jj)] = h

            def stage2(kt):
                p, t = divmod(kt, NK)
                if t == 0:
                    accq[p] = accs.tile([128, BW], F32, name="acc", tag="acc")
                acc = accq[p]
                st, sp = t == 0, t == NK - 1
                hA = hq.pop((p, 2 * t))
                hB = hq.pop((p, 2 * t + 1))
                wA = w2s[:, bass.ds(32 * (2 * t), 32)]
                wB = w2s[:, bass.ds(32 * (2 * t + 1), 32)]
                asl = bass.ds(0, BW)
                bsl = bass.ds(BW, BW)
                # 4 concurrent K=128, M=32 matmuls into distinct col strips
                nc.tensor.matmul(acc[0:32, :], wA, hA[:, asl], start=st, stop=sp,
                                 skip_group_check=True, tile_position=(0, 0))
                nc.tensor.matmul(acc[64:96, :], wA, hA[:, bsl], start=st, stop=sp,
                                 skip_group_check=True, tile_position=(0, 64))
                nc.tensor.matmul(acc[32:64, :], wB, hB[:, asl], start=st, stop=sp,
                                 skip_group_check=True, tile_position=(0, 32))
                nc.tensor.matmul(acc[96:128, :], wB, hB[:, bsl], start=st, stop=sp,
                                 skip_group_check=True, tile_position=(0, 96))
                if sp:
                    acc = accq.pop(p)
                    so = sop.tile([128, BW], DT)
                    if p == NPAIR - 1:
                        # last pair: both engines drained; split the copy and
                        # fan the final store across three queues
                        nc.vector.tensor_copy(so[:, 0:256], acc[:, 0:256])
                        nc.scalar.activation(so[:, 256:BW], acc[:, 256:BW],
                                             mybir.ActivationFunctionType.Copy)
                        nc.sync.dma_start(out[0:32, bass.ds(p * BW, BW)],
                                          so[0:32, :])
                        nc.gpsimd.dma_start(out[32:64, bass.ds(p * BW, BW)],
                                            so[32:64, :])
                        nc.scalar.dma_start(
                            out[0:32, bass.ds(HALF + p * BW, BW)], so[64:96, :])
                        nc.sync.dma_start(
                            out[32:64, bass.ds(HALF + p * BW, BW)],
                            so[96:128, :])
                    else:
                        nc.scalar.activation(so[:], acc[:],
                                             mybir.ActivationFunctionType.Copy)
                        nc.sync.dma_start(out[:, bass.ds(p * BW, BW)],
                                          so[0:64, :])
                        nc.sync.dma_start(out[:, bass.ds(HALF + p * BW, BW)],
                                          so[64:128, :])

            for kt in range(NKT + DELAY):
                if kt < NKT:
                    stage1(kt)
                if kt >= DELAY:
                    stage2(kt - DELAY)

    nc.compile()
    return nc


# dram-out row r holds node PERM[r] (stripe-packed stage-2 layout)
PERM = np.array([4 * ((p % 32) // 2) + 2 * (p // 32) + (p % 2)
                 for p in range(64)])


def kernel(t, inputs, W1, W2, adjacency, discrete_mask, **_ignored):
    global LAST_EXEC_NS
    inputs = np.asarray(inputs, np.float32)
    W1 = np.asarray(W1, np.float32)
    W2 = np.asarray(W2, np.float32)
    adjacency = np.asarray(adjacency, np.float32)
    discrete_mask = np.asarray(discrete_mask)

    # ---- host-side input prep: straight-through gate is pure data prep ----
    x = np.where(discrete_mask[None, :], (inputs > 0).astype(np.float32), inputs)

    # ---- host-side weight folding / layout ----
    M = adjacency.copy()
    one_hot_last = np.zeros(N, np.float32)
    one_hot_last[-1] = 1.0
    M[-1] = M[-1] * one_hot_last
    W1m = W1 * M[:, None, :]                      # [N, H, N]
    # fold |W2| into W1 rows: relu(|w| z) == |w| relu(z); signs go to stage 2
    W1e = W1m * np.abs(W2)[:, :, None]
    w1t = np.ascontiguousarray(W1e.reshape(IH, N).T)   # [N, IH]

    sgn = np.sign(W2).astype(np.float32)          # [N, H]
    # compact class-major sign table; the kernel scatters it into w2s
    # (w2s[64u:64u+64, 66*t + 32*pbit + u] = sgn[4t + 2*pbit + u])
    w2c = np.zeros((128, 64), np.float32)
    for tt in range(16):
        for pbit in range(2):
            for u in range(2):
                node = 4 * tt + 2 * pbit + u
                cls = 2 * pbit + u
                w2c[64 * u : 64 * u + 64, 16 * cls + tt] = sgn[node]

    xt = np.ascontiguousarray(x.T)                # [N, B]

    if 0 not in _compiled:
        _compiled[0] = _build_module()
    nc = _compiled[0]

    w1t_d = w1t.astype(NP_DT)
    w2c_d = w2c.astype(NP_DT)
    xt_d = xt.astype(NP_DT)
    in_maps = [
        {
            "xt": np.ascontiguousarray(xt_d[:, c * BL : (c + 1) * BL]),
            "w1": w1t_d,
            "w2c": w2c_d,
        }
        for c in range(N_CORES)
    ]

    trace = bool(int(os.environ.get("KERNEL_TRACE", "0")))
    res = run_bass_kernel_spmd(
        nc, in_maps, core_ids=list(range(N_CORES)), trace=trace
    )
    if trace:
        LAST_EXEC_NS = res.exec_time_ns
        globals()["LAST_RESULT"] = res

    outT = np.concatenate(
        [res.results[c]["out"] for c in range(N_CORES)], axis=1
    ).astype(np.float32)
    # rows are stripe-permuted: row r holds node PERM[r]
    unperm = np.empty_like(outT)
    unperm[PERM] = outT
    return np.ascontiguousarray(unperm.T)


# revision 13
# speedup vs baseline: 2.6496x; 2.6496x over previous
"""Trainium2 Bass kernel for nn_CausalDerivative (per-node MLP stack).

Computation (reference):
    x = where(discrete_mask, (inputs > 0), inputs)          # straight-through gate
    W1m = W1 * M[:, None, :]   (M = adjacency, last row one-hot @ last col)
    h = relu(einsum('bn,ihn->bih', x, W1m))                 # [B, N, H]
    out = einsum('bih,ih->bi', h, W2)                       # [B, N]

Strategy: pure data-parallel over 8 NeuronCores (batch sharded 4096/core).

Per core the kernel is paced by the relu eviction of the 16.8M-element z
intermediate (PSUM f32 -> SBUF bf16); on TRN2 only DVE (0.96 GHz) and ACT
(1.2 GHz) can read PSUM at 1 f32/cycle/lane, so ~70us/core is the floor.
Design (v2, ring conveyor):

  - straight-through gate and weight folding host-side; |W2| folded into
    W1 so stage-2 weights are signs; the sign matrix w2s is built ON CHIP
    (memset + 4 strided copies from a 16KB compact DMA) instead of a
    256KB DMA of mostly zeros.
  - stage 1: per 128-unit chunk, 2 block-diagonal K=64/M=128 matmuls
    (tile rows 0/64) each fill ONE PSUM bank [128u, 512b] -- half the MM
    and LDWEIGHTS instructions of the 4-quadrant scheme.
  - PSUM is a 7-bank ring (the 8th bank is the single-buffered stage-2
    accumulator).  Chunk g occupies banks (2g mod 7, 2g+1 mod 7); the
    eviction is ONE strided 2D-AP op over the pair (wrap pair (6,0) is
    filled A->0, B->6 so the AP stays ascending).  Ring depth 3.5 chunks
    decouples fills from the evict critical cycle (vs 3 slots before).
  - evictions alternate DVE / ACT per chunk; first two chunks split
    across both engines to start the conveyor early.
  - ~12 warm-up matmuls on a memset scratch trip the PE HAM throttle
    (cold 1.2GHz -> warm 2.4GHz) before the real fills arrive.
  - DMA: first wave = exactly k-tile 0's needs on 3 queues; w1 then
    streams in progressive column pieces so chunk j never waits on a
    monolithic transfer; sx second half last (needed 16 k-tiles in).
  - stage 2: per chunk pair, 4 concurrent K=128/M=32 col-strip matmuls
    accumulate into the acc bank over 16 pairs; acc -> SBUF copy on ACT
    (split DVE/ACT for the last tile) then DMA out.
  - node rows come out stripe-permuted; the host unpermutes.
"""

import os
import numpy as np

import concourse.bass as bass
import concourse.tile as tile
from concourse import mybir, bacc
from concourse.bass import ts
from concourse.bass_utils import run_bass_kernel_spmd

B, N, H = 32768, 64, 64
IH = N * H                    # 4096 hidden units total
N_CORES = 8
BL = B // N_CORES             # 4096 batch rows per core
HALF = BL // 2                # 2048 (batch half per SBUF partition group)
BW = 512                      # batch tile width (PE moving free dim)
NPAIR = HALF // BW            # 4 batch tiles per core
NCHUNK = IH // 128            # 32 ih chunks of 128 units (2 nodes each)
NK = NCHUNK // 2              # 16 chunk-pairs (k-tiles) per batch tile
RING = 7                      # PSUM ring banks (bank 8 = stage-2 acc)
NWARM = 12                    # HAM warm-up matmuls

F32 = mybir.dt.float32
BF16 = mybir.dt.bfloat16
DT = BF16
import ml_dtypes
NP_DT = ml_dtypes.bfloat16

LAST_EXEC_NS = None

_compiled = {}


def _ring_banks(g):
    """Chunk g -> (bankA, bankB, dynslice over the pair)."""
    base = (2 * g) % RING
    if base <= RING - 2:
        return base, base + 1, bass.ds(base, 2)
    # wrap: fill A into bank 0, B into bank 6 so ds(0,2,step=6) reads A|B
    return 0, RING - 1, bass.ds(0, 2, RING - 1)


def _build_module():
    """Emit the per-core Bass module (same program for all 8 cores)."""
    nc = bacc.Bacc("TRN2", target_bir_lowering=False, debug=False)
    xt = nc.dram_tensor("xt", [N, BL], DT, kind="ExternalInput").ap()
    w1 = nc.dram_tensor("w1", [N, IH], DT, kind="ExternalInput").ap()
    w2c = nc.dram_tensor("w2c", [128, 64], DT, kind="ExternalInput").ap()
    out = nc.dram_tensor("out", [N, BL], DT, kind="ExternalOutput").ap()

    DELAY = 3                 # stage-2 lags stage-1 by DELAY k-tiles
    NKT = NPAIR * NK          # 64 k-tiles total

    with tile.TileContext(nc) as tc:
        with (
            tc.tile_pool(name="consts", bufs=1) as consts,
            tc.tile_pool(name="hp", bufs=10) as hp,
            tc.tile_pool(name="so", bufs=2) as sop,
            tc.tile_pool(name="ps", bufs=3, space="PSUM") as psp,
            tc.tile_pool(name="accs", bufs=2, space="PSUM") as accs,
        ):
            sx = consts.tile([128, HALF], DT)
            w1s = consts.tile([128, IH], DT)
            w2s = consts.tile([128, NCHUNK * 32], DT)
            w2stage = consts.tile([128, 64], DT)
            warm = consts.tile([64, 192], DT)

            # ---- HAM warm-up: keep PE busy from t=0 so the clock gate
            # releases (1.2 -> 2.4 GHz) before the real fills arrive.
            nc.gpsimd.memset(warm[:], 0.0)
            wps = psp.tile([128, 2 * BW], F32, name="ps", tag="ps")
            for _ in range(NWARM):
                nc.tensor.matmul(wps[0:64, 0:128], warm[0:64, 0:64],
                                 warm[0:64, 64:192])

            # ---- startup DMA.  Wave 1: exactly what chunk 0 needs.
            nc.scalar.dma_start(w1s[0:64, 0:256], w1[:, 0:256])
            nc.scalar.dma_start(w1s[64:128, 0:256], w1[:, 0:256])
            nc.sync.dma_start(sx[0:64, 0:BW], xt[:, 0:BW])
            nc.gpsimd.dma_start(sx[64:128, 0:BW], xt[:, HALF : HALF + BW])
            nc.scalar.dma_start(w2stage[:, :], w2c[:, :])
            # Wave 2: w1 in progressive 512-col pieces (A half on sync, B half
            # on gpsimd) so chunk j never waits on a monolithic transfer.
            w1cuts = [256] + list(range(768, IH, 512)) + [IH]
            for i in range(len(w1cuts) - 1):
                c0, c1 = w1cuts[i], w1cuts[i + 1]
                nc.sync.dma_start(w1s[0:64, c0:c1], w1[:, c0:c1])
                nc.gpsimd.dma_start(w1s[64:128, c0:c1], w1[:, c0:c1])
                if i == 1:
                    # zero the sign matrix once the early triggers are posted
                    nc.gpsimd.memset(w2s[:, :], 0.0)
            # Wave 3: rest of sx (needed from k-tile 16 onward).
            sxcuts = [BW, 1280, HALF]
            for i in range(len(sxcuts) - 1):
                c0, c1 = sxcuts[i], sxcuts[i + 1]
                nc.sync.dma_start(sx[0:64, c0:c1], xt[:, c0:c1])
                nc.gpsimd.dma_start(sx[64:128, c0:c1], xt[:, HALF + c0 : HALF + c1])
            # Build the sparse stage-2 sign matrix on chip: 4 strided
            # copies scatter the compact class-major signs into w2s.
            for pbit in range(2):
                for u in range(2):
                    cls = 2 * pbit + u
                    nc.gpsimd.tensor_copy(
                        w2s[64 * u : 64 * u + 64, bass.ds(32 * pbit + u, 16, 66)],
                        w2stage[64 * u : 64 * u + 64, bass.ds(16 * cls, 16)],
                    )

            hq = {}
            accq = {}

            def stage1(kt):
                p, t = divmod(kt, NK)
                bs = ts(p, BW)
                for u2 in range(2):
                    g = 2 * kt + u2           # global chunk index
                    jj = 2 * t + u2           # chunk within batch tile
                    cw = bass.ds(128 * jj, 128)
                    ps = psp.tile([128, 2 * BW], F32, name="ps", tag="ps")
                    # block-diagonal fill: 2 concurrent K=64/M=128 matmuls
                    # (tile rows 0/64), one PSUM bank each
                    nc.tensor.matmul(ps[:, 0:BW], w1s[0:64, cw],
                                     sx[0:64, bs], tile_position=(0, 0))
                    nc.tensor.matmul(ps[:, BW : 2 * BW], w1s[64:128, cw],
                                     sx[64:128, bs], tile_position=(64, 0))
                    h = hp.tile([128, 2 * BW], DT)
                    if g < 2:
                        # conveyor start: split across both engines
                        nc.vector.tensor_scalar_max(h[:, 0:BW], ps[:, 0:BW],
                                                    0.0)
                        nc.scalar.activation(h[:, BW : 2 * BW],
                                             ps[:, BW : 2 * BW],
                                             mybir.ActivationFunctionType.Relu)
                    elif g % 2 == 0:
                        nc.vector.tensor_scalar_max(h[:], ps[:], 0.0)
                    else:
                        nc.scalar.activation(h[:], ps[:],
                                             mybir.ActivationFunctionType.Relu)
                    hq[(p, jj)] = h

            def stage2(kt):
                p, t = divmod(kt, NK)
                if t == 0:
                    accq[p] = accs.tile([128, BW], F32, name="acc", tag="acc")
                acc = accq[p]
                st, sp = t == 0, t == NK - 1
                hA = hq.pop((p, 2 * t))
                hB = hq.pop((p, 2 * t + 1))
                wA = w2s[:, bass.ds(32 * (2 * t), 32)]
                wB = w2s[:, bass.ds(32 * (2 * t + 1), 32)]
                asl = bass.ds(0, BW)
                bsl = bass.ds(BW, BW)
                # 4 concurrent K=128, M=32 matmuls into distinct col strips
                nc.tensor.matmul(acc[0:32, :], wA, hA[:, asl], start=st, stop=sp,
                                 skip_group_check=True, tile_position=(0, 0))
                nc.tensor.matmul(acc[64:96, :], wA, hA[:, bsl], start=st, stop=sp,
                                 skip_group_check=True, tile_position=(0, 64))
                nc.tensor.matmul(acc[32:64, :], wB, hB[:, asl], start=st, stop=sp,
                                 skip_group_check=True, tile_position=(0, 32))
                nc.tensor.matmul(acc[96:128, :], wB, hB[:, bsl], start=st, stop=sp,
                                 skip_group_check=True, tile_position=(0, 96))
                if sp:
                    acc = accq.pop(p)
                    so = sop.tile([128, BW], DT)
                    if p == NPAIR - 1:
                        # last pair: both engines drained; split the copy and
                        # fan the final store across three queues
                        nc.vector.tensor_copy(so[:, 0:256], acc[:, 0:256])
                        nc.scalar.activation(so[:, 256:BW], acc[:, 256:BW],
                                             mybir.ActivationFunctionType.Copy)
                        nc.sync.dma_start(out[0:32, bass.ds(p * BW, BW)],
                                          so[0:32, :])
                        nc.gpsimd.dma_start(out[32:64, bass.ds(p * BW, BW)],
                                            so[32:64, :])
                        nc.scalar.dma_start(
                            out[0:32, bass.ds(HALF + p * BW, BW)], so[64:96, :])
                        nc.sync.dma_start(
                            out[32:64, bass.ds(HALF + p * BW, BW)],
                            so[96:128, :])
                    else:
                        nc.scalar.activation(so[:], acc[:],
                                             mybir.ActivationFunctionType.Copy)
                        nc.sync.dma_start(out[:, bass.ds(p * BW, BW)],
                                          so[0:64, :])
                        nc.sync.dma_start(out[:, bass.ds(HALF + p * BW, BW)],
                                          so[64:128, :])

            for kt in range(NKT + DELAY):
                if kt < NKT:
                    stage1(kt)
                if kt >= DELAY:
                    stage2(kt - DELAY)

    nc.compile()
    return nc


# dram-out row r holds node PERM[r] (stripe-packed stage-2 layout)
PERM = np.array([4 * ((p % 32) // 2) + 2 * (p // 32) + (p % 2)
                 for p in range(64)])


def kernel(t, inputs, W1, W2, adjacency, discrete_mask, **_ignored):
    global LAST_EXEC_NS
    inputs = np.asarray(inputs, np.float32)
    W1 = np.asarray(W1, np.float32)
    W2 = np.asarray(W2, np.float32)
    adjacency = np.asarray(adjacency, np.float32)
    discrete_mask = np.asarray(discrete_mask)

    # ---- host-side input prep: straight-through gate is pure data prep ----
    x = np.where(discrete_mask[None, :], (inputs > 0).astype(np.float32), inputs)

    # ---- host-side weight folding / layout ----
    M = adjacency.copy()
    one_hot_last = np.zeros(N, np.float32)
    one_hot_last[-1] = 1.0
    M[-1] = M[-1] * one_hot_last
    W1m = W1 * M[:, None, :]                      # [N, H, N]
    # fold |W2| into W1 rows: relu(|w| z) == |w| relu(z); signs go to stage 2
    W1e = W1m * np.abs(W2)[:, :, None]
    w1t = np.ascontiguousarray(W1e.reshape(IH, N).T)   # [N, IH]

    sgn = np.sign(W2).astype(np.float32)          # [N, H]
    # compact class-major sign table; the kernel scatters it into w2s
    # (w2s[64u:64u+64, 66*t + 32*pbit + u] = sgn[4t + 2*pbit + u])
    w2c = np.zeros((128, 64), np.float32)
    for tt in range(16):
        for pbit in range(2):
            for u in range(2):
                node = 4 * tt + 2 * pbit + u
                cls = 2 * pbit + u
                w2c[64 * u : 64 * u + 64, 16 * cls + tt] = sgn[node]

    xt = np.ascontiguousarray(x.T)                # [N, B]

    if 0 not in _compiled:
        _compiled[0] = _build_module()
    nc = _compiled[0]

    w1t_d = w1t.astype(NP_DT)
    w2c_d = w2c.astype(NP_DT)
    xt_d = xt.astype(NP_DT)
    in_maps = [
        {
            "xt": np.ascontiguousarray(xt_d[:, c * BL : (c + 1) * BL]),
            "w1": w1t_d,
            "w2c": w2c_d,
        }
        for c in range(N_CORES)
    ]

    trace = bool(int(os.environ.get("KERNEL_TRACE", "0")))
    res = run_bass_kernel_spmd(
        nc, in_maps, core_ids=list(range(N_CORES)), trace=trace
    )
    if trace:
        LAST_EXEC_NS = res.exec_time_ns
        globals()["LAST_RESULT"] = res

    outT = np.concatenate(
        [res.results[c]["out"] for c in range(N_CORES)], axis=1
    ).astype(np.float32)
    # rows are stripe-permuted: row r holds node PERM[r]
    unperm = np.empty_like(outT)
    unperm[PERM] = outT
    return np.ascontiguousarray(unperm.T)
